# revision 7
# baseline (speedup 1.0000x reference)
"""Trainium2 Bass kernel for a single transformer decoder layer.

Problem: nn_DecoderLayer — B=8, S=1024, D=768, H=12, DFF=3072, causal MHA +
FFN with two LayerNorms (post-norm residual).

Strategy: pure data-parallel over batch — one batch element per NeuronCore
(8 cores).  All matmuls in bf16 on the TensorEngine (fp32 PSUM accumulate).
Activations kept in a transposed layout where that lets matmul outputs feed
the next matmul without on-chip transposes:

  qT/kT  = (x@wq)^T computed directly as wq.T@x = matmul(lhsT=wq, rhs=xT)
  logitsT[k,q] = matmul(lhsT=kT_head_slice, rhs=qT_head_slice)   (K=64)
  expT = exp(logitsT*scale)  — elementwise, so expT == (softmax numerator)^T
  ctx[q,:]  = matmul(lhsT=expT, rhs=[v | ones])  -> col 64 gives softmax
              denominator per q-row; normalize via ACT Copy w/ scale AP.
  attn_out  = matmul(lhsT=ctxT, rhs=wo)  (ctx transposed on PE)
  hT        = relu(matmul(lhsT=w1, rhs=out1T) + b1)  (b1 per-partition)
  ffn       = matmul(lhsT=hT, rhs=w2)
  LayerNorms in fp32 on DVE via bn_stats/bn_aggr (free-dim = feature dim).

Causal mask: only the lower-triangular k-tiles are computed; the diagonal
128x128 blocks are multiplied by a 0/1 triangular mask after exp.
"""

import sys

import numpy as np

if "/opt/trn_rl_repo" not in sys.path:
    sys.path.insert(0, "/opt/trn_rl_repo")

import ml_dtypes

BF16 = ml_dtypes.bfloat16

D = 768
S = 1024
H = 12
DH = 64
F = 3072
B = 8
KT = D // 128  # 6
ST = S // 128  # 8
FT = F // 128  # 24
N_CORES = 8
EPS = 1e-6
ATT_SCALE = 1.0 / 8.0  # 1/sqrt(64)

_BUILD_CACHE = {}


def _build_nc():
    """Build the single-core Bass program (same program runs SPMD on 8 cores)."""
    from contextlib import ExitStack

    import concourse.bass as bass
    import concourse.tile as tile
    from concourse import bacc, mybir

    f32 = mybir.dt.float32
    bf16 = mybir.dt.bfloat16
    AF = mybir.ActivationFunctionType
    ALU = mybir.AluOpType

    nc = bacc.Bacc(
        "TRN2", target_bir_lowering=False, debug=False, num_devices=N_CORES
    )

    # ---- DRAM parameters (per-core inputs) ----
    xT_d = nc.declare_dram_parameter("xT", [D, S], bf16, isOutput=False)
    x_d = nc.declare_dram_parameter("x", [S, D], f32, isOutput=False)
    wq_d = nc.declare_dram_parameter("wq", [D, D], bf16, isOutput=False)
    wk_d = nc.declare_dram_parameter("wk", [D, D], bf16, isOutput=False)
    wv_d = nc.declare_dram_parameter("wv", [D, D], bf16, isOutput=False)
    wo_d = nc.declare_dram_parameter("wo", [D, D], bf16, isOutput=False)
    w1_d = nc.declare_dram_parameter("w1", [D, F], bf16, isOutput=False)
    w2_d = nc.declare_dram_parameter("w2", [F, D], bf16, isOutput=False)
    bq_d = nc.declare_dram_parameter("bq", [D], f32, isOutput=False)
    bk_d = nc.declare_dram_parameter("bk", [D], f32, isOutput=False)
    b1_d = nc.declare_dram_parameter("b1", [F], f32, isOutput=False)
    # free-dim-broadcast vectors, pre-cast to bf16 on host
    bv_d = nc.declare_dram_parameter("bv", [D], bf16, isOutput=False)
    bo_d = nc.declare_dram_parameter("bo", [D], bf16, isOutput=False)
    b2_d = nc.declare_dram_parameter("b2", [D], bf16, isOutput=False)
    g1_d = nc.declare_dram_parameter("g1", [D], bf16, isOutput=False)
    be1_d = nc.declare_dram_parameter("be1", [D], bf16, isOutput=False)
    g2_d = nc.declare_dram_parameter("g2", [D], bf16, isOutput=False)
    be2_d = nc.declare_dram_parameter("be2", [D], bf16, isOutput=False)
    tri_d = nc.declare_dram_parameter("trimask", [128, 128], bf16, isOutput=False)
    idb_d = nc.declare_dram_parameter("identb", [128, 128], bf16, isOutput=False)
    out_d = nc.declare_dram_parameter("out", [S, D], f32, isOutput=True)

    with ExitStack() as ctx:
        tc = ctx.enter_context(tile.TileContext(nc))

        # ---- SBUF pools ----
        wpool = ctx.enter_context(tc.tile_pool(name="wpool", bufs=5))
        xTp = ctx.enter_context(tc.tile_pool(name="xTp", bufs=1))
        qkvp = ctx.enter_context(tc.tile_pool(name="qkvp", bufs=3))
        ctxp = ctx.enter_context(tc.tile_pool(name="ctxp", bufs=1))
        out1p = ctx.enter_context(tc.tile_pool(name="out1p", bufs=1))
        bigp = ctx.enter_context(tc.tile_pool(name="bigp", bufs=24))
        bcp = ctx.enter_context(tc.tile_pool(name="bcp", bufs=1))
        strp = ctx.enter_context(tc.tile_pool(name="strp", bufs=2))
        tinyp = ctx.enter_context(tc.tile_pool(name="tinyp", bufs=4))
        psA = ctx.enter_context(tc.tile_pool(name="psA", bufs=4, space="PSUM"))
        psC = ctx.enter_context(tc.tile_pool(name="psC", bufs=2, space="PSUM"))
        psT = ctx.enter_context(tc.tile_pool(name="psT", bufs=2, space="PSUM"))

        # ---- constant / weight loads ----
        xT_sb = xTp.tile([128, KT, S], bf16, name="xT_sb", tag="xbig")
        nc.sync.dma_start(out=xT_sb[:], in_=xT_d[:].rearrange("(k p) n -> p k n", p=128))

        wq_sb = wpool.tile([128, KT, D], bf16, name="wq_sb", tag="w")
        nc.sync.dma_start(out=wq_sb[:], in_=wq_d[:].rearrange("(k p) n -> p k n", p=128))
        wk_sb = wpool.tile([128, KT, D], bf16, name="wk_sb", tag="w")
        nc.sync.dma_start(out=wk_sb[:], in_=wk_d[:].rearrange("(k p) n -> p k n", p=128))
        wv_sb = wpool.tile([128, KT, D], bf16, name="wv_sb", tag="w")
        nc.sync.dma_start(out=wv_sb[:], in_=wv_d[:].rearrange("(k p) n -> p k n", p=128))
        wo_sb = wpool.tile([128, KT, D], bf16, name="wo_sb", tag="w")
        nc.sync.dma_start(out=wo_sb[:], in_=wo_d[:].rearrange("(k p) n -> p k n", p=128))

        bq_sb = bcp.tile([128, KT], f32, name="bq_sb", tag="bq")
        nc.sync.dma_start(out=bq_sb[:], in_=bq_d[:].rearrange("(t p) -> p t", p=128))
        bk_sb = bcp.tile([128, KT], f32, name="bk_sb", tag="bk")
        nc.sync.dma_start(out=bk_sb[:], in_=bk_d[:].rearrange("(t p) -> p t", p=128))
        b1_sb = bcp.tile([128, FT], f32, name="b1_sb", tag="b1")
        nc.sync.dma_start(out=b1_sb[:], in_=b1_d[:].rearrange("(t p) -> p t", p=128))

        def bcast128(dram, name):
            t = bcp.tile([128, D], bf16, name=name, tag=name)
            src = bass.AP(
                tensor=dram[:].tensor,
                offset=dram[:].offset,
                ap=[[0, 128]] + list(dram[:].ap),
            )
            nc.gpsimd.dma_start(out=t[:], in_=src)
            return t

        bv_bc = bcast128(bv_d, "bv_bc")
        bo_bc = bcast128(bo_d, "bo_bc")
        b2_bc = bcast128(b2_d, "b2_bc")
        g1_bc = bcast128(g1_d, "g1_bc")
        be1_bc = bcast128(be1_d, "be1_bc")
        g2_bc = bcast128(g2_d, "g2_bc")
        be2_bc = bcast128(be2_d, "be2_bc")

        tri_sb = bcp.tile([128, 128], bf16, name="tri_sb", tag="tri")
        nc.sync.dma_start(out=tri_sb[:], in_=tri_d[:])
        idb_sb = bcp.tile([128, 128], bf16, name="idb_sb", tag="idb")
        nc.sync.dma_start(out=idb_sb[:], in_=idb_d[:])

        eps_sb = tinyp.tile([128, 1], f32, name="eps_sb", tag="eps", bufs=1)
        nc.vector.memset(eps_sb[:], EPS)

        # ---- phase 1: QKV projections ----
        qT_sb = qkvp.tile([128, KT, S], bf16, name="qT_sb", tag="qkv")
        kT_sb = qkvp.tile([128, KT, S], bf16, name="kT_sb", tag="qkv")
        # v with a ones column appended per head: [s_tile, head, 65]
        v_aug = qkvp.tile([128, ST, H, DH + 1], bf16, name="v_aug", tag="qkv")
        nc.vector.memset(v_aug[:, :, :, DH : DH + 1], 1.0)

        # qT / kT: out[dout, s] accumulated over k; bias is per-partition
        for w_sb, b_sb, o_sb in ((wq_sb, bq_sb, qT_sb), (wk_sb, bk_sb, kT_sb)):
            for t in range(KT):
                for c in range(2):
                    ps = psA.tile([128, 512], f32, name="ps_mm", tag="mm")
                    for k in range(KT):
                        nc.tensor.matmul(
                            ps[:],
                            lhsT=w_sb[:, k, 128 * t : 128 * (t + 1)],
                            rhs=xT_sb[:, k, 512 * c : 512 * (c + 1)],
                            start=(k == 0),
                            stop=(k == KT - 1),
                        )
                    nc.scalar.activation(
                        out=o_sb[:, t, 512 * c : 512 * (c + 1)],
                        in_=ps[:],
                        func=AF.Identity,
                        bias=b_sb[:, t : t + 1],
                        scale=1.0,
                    )

        # v: out[s, dout] straight; bias broadcast over partitions
        for m in range(ST):
            for c in range(2):
                ps = psA.tile([128, 384], f32, name="ps_mm", tag="mm")
                for k in range(KT):
                    nc.tensor.matmul(
                        ps[:],
                        lhsT=xT_sb[:, k, 128 * m : 128 * (m + 1)],
                        rhs=wv_sb[:, k, 384 * c : 384 * (c + 1)],
                        start=(k == 0),
                        stop=(k == KT - 1),
                    )
                nc.vector.tensor_add(
                    out=v_aug[:, m, 6 * c : 6 * (c + 1), 0:DH],
                    in0=ps[:].rearrange("p (h d) -> p h d", h=6),
                    in1=bv_bc[:, 384 * c : 384 * (c + 1)].rearrange(
                        "p (h d) -> p h d", h=6
                    ),
                )

        # ---- phase 2: attention (software-pipelined across heads) ----
        ctx_sb = ctxp.tile([128, ST, D], bf16, name="ctx_sb", tag="ctx")

        def head_slices(h):
            p0 = (h % 2) * DH
            t = h // 2
            return p0, t

        def emit_logits(h):
            p0, t = head_slices(h)
            tiles = []
            for i in range(ST):
                et = bigp.tile([128, S], bf16, name=f"expT_{h}_{i}", tag="big")
                tiles.append(et)
                for c in range(i // 4, 2):
                    ps = psA.tile([128, 512], f32, name="ps_mm", tag="mm")
                    nc.tensor.matmul(
                        ps[:],
                        lhsT=kT_sb[p0 : p0 + DH, t, 128 * i : 128 * (i + 1)],
                        rhs=qT_sb[p0 : p0 + DH, t, 512 * c : 512 * (c + 1)],
                        start=True,
                        stop=True,
                    )
                    nc.scalar.activation(
                        out=et[:, 512 * c : 512 * (c + 1)],
                        in_=ps[:],
                        func=AF.Exp,
                        scale=ATT_SCALE,
                    )
                # causal mask on the diagonal 128x128 block
                nc.vector.tensor_mul(
                    out=et[:, 128 * i : 128 * (i + 1)],
                    in0=et[:, 128 * i : 128 * (i + 1)],
                    in1=tri_sb[:],
                )
            return tiles

        def emit_ctx(h, tiles):
            p0, t = head_slices(h)
            for m in range(ST):
                pc = psC.tile([128, DH + 1], f32, name="ps_ctx", tag="ctx")
                for i in range(m + 1):
                    nc.tensor.matmul(
                        pc[:],
                        lhsT=tiles[i][:, 128 * m : 128 * (m + 1)],
                        rhs=v_aug[:, i, h, :],
                        start=(i == 0),
                        stop=(i == m),
                    )
                rc = tinyp.tile([128, 1], f32, name="recip", tag="recip")
                nc.vector.reciprocal(out=rc[:], in_=pc[:, DH : DH + 1])
                nc.scalar.activation(
                    out=ctx_sb[:, m, DH * h : DH * (h + 1)],
                    in_=pc[:, 0:DH],
                    func=AF.Copy,
                    scale=rc[:],
                )

        prev = None
        for h in range(H):
            cur = emit_logits(h)
            if prev is not None:
                emit_ctx(h - 1, prev)
            prev = cur
        emit_ctx(H - 1, prev)

        # ---- phase 3: attn output projection + residual + LN1 ----
        ctxT_sb = xTp.tile([128, KT, S], bf16, name="ctxT_sb", tag="xbig")
        out1_sb = out1p.tile([128, ST, D], bf16, name="out1_sb", tag="out1")
        out1T_sb = qkvp.tile([128, KT, S], bf16, name="out1T_sb", tag="qkv")

        def emit_ctxT(m):
            for k in range(KT):
                pt = psT.tile([128, 128], bf16, name="ps_tr", tag="tr")
                nc.tensor.transpose(
                    pt[:], ctx_sb[:, m, 128 * k : 128 * (k + 1)], idb_sb[:]
                )
                nc.scalar.activation(
                    out=ctxT_sb[:, k, 128 * m : 128 * (m + 1)],
                    in_=pt[:],
                    func=AF.Copy,
                )

        def emit_ln(res, mean_var_src, out_ap, g_bc, be_bc):
            stats = tinyp.tile([128, 3, 6], f32, name="stats", tag="stats")
            for sg in range(3):
                nc.vector.bn_stats(
                    out=stats[:, sg, :], in_=mean_var_src[:, 256 * sg : 256 * (sg + 1)]
                )
            mv = tinyp.tile([128, 2], f32, name="mv", tag="mv")
            nc.vector.bn_aggr(out=mv[:], in_=stats[:])
            std = tinyp.tile([128, 1], f32, name="std", tag="std")
            nc.scalar.activation(
                out=std[:], in_=mv[:, 1:2], func=AF.Sqrt, bias=eps_sb[:], scale=1.0
            )
            rstd = tinyp.tile([128, 1], f32, name="rstd", tag="rstd")
            nc.vector.reciprocal(out=rstd[:], in_=std[:])
            nc.vector.tensor_scalar(
                out=out_ap,
                in0=res[:],
                scalar1=mv[:, 0:1],
                scalar2=rstd[:],
                op0=ALU.subtract,
                op1=ALU.mult,
            )
            nc.gpsimd.tensor_mul(out=out_ap, in0=out_ap, in1=g_bc[:])
            nc.gpsimd.tensor_add(out=out_ap, in0=out_ap, in1=be_bc[:])

        emit_ctxT(0)
        emit_ctxT(1)
        for m in range(ST):
            x_m = strp.tile([128, D], f32, name="x_m", tag="xs")
            nc.sync.dma_start(out=x_m[:], in_=x_d[128 * m : 128 * (m + 1), :])
            res = strp.tile([128, D], f32, name="res1", tag="res")
            for n0 in range(2):
                ps = psA.tile([128, 384], f32, name="ps_mm", tag="mm")
                for k in range(KT):
                    nc.tensor.matmul(
                        ps[:],
                        lhsT=ctxT_sb[:, k, 128 * m : 128 * (m + 1)],
                        rhs=wo_sb[:, k, 384 * n0 : 384 * (n0 + 1)],
                        start=(k == 0),
                        stop=(k == KT - 1),
                    )
                nc.vector.tensor_add(
                    out=res[:, 384 * n0 : 384 * (n0 + 1)],
                    in0=ps[:],
                    in1=x_m[:, 384 * n0 : 384 * (n0 + 1)],
                )
            if m + 2 < ST:
                emit_ctxT(m + 2)
            nc.gpsimd.tensor_add(out=res[:], in0=res[:], in1=bo_bc[:])
            emit_ln(res, res, out1_sb[:, m, :], g1_bc, be1_bc)

        # transpose out1 -> out1T for the FFN
        for m in range(ST):
            for k in range(KT):
                pt = psT.tile([128, 128], bf16, name="ps_tr", tag="tr")
                nc.tensor.transpose(
                    pt[:], out1_sb[:, m, 128 * k : 128 * (k + 1)], idb_sb[:]
                )
                nc.scalar.activation(
                    out=out1T_sb[:, k, 128 * m : 128 * (m + 1)],
                    in_=pt[:],
                    func=AF.Copy,
                )

        # ---- phase 4: FFN first layer (relu(out1 @ w1 + b1), transposed) ----
        w1_sbs = []
        for s_ in range(3):
            w1s = wpool.tile([128, 2, F], bf16, name=f"w1_sb{s_}", tag="w")
            nc.sync.dma_start(
                out=w1s[:],
                in_=w1_d[256 * s_ : 256 * (s_ + 1), :].rearrange(
                    "(kk p) n -> p kk n", p=128
                ),
            )
            w1_sbs.append(w1s)
        w2_sbs = []
        for g in range(3):
            w2s = wpool.tile([128, 8, D], bf16, name=f"w2_sb{g}", tag="w")
            nc.sync.dma_start(
                out=w2s[:],
                in_=w2_d[1024 * g : 1024 * (g + 1), :].rearrange(
                    "(kk p) n -> p kk n", p=128
                ),
            )
            w2_sbs.append(w2s)

        hts = []
        for mf in range(FT):
            ht = bigp.tile([128, S], bf16, name=f"hT_{mf}", tag="big")
            hts.append(ht)
            for c in range(2):
                ps = psA.tile([128, 512], f32, name="ps_mm", tag="mm")
                for k in range(KT):
                    nc.tensor.matmul(
                        ps[:],
                        lhsT=w1_sbs[k // 2][:, k % 2, 128 * mf : 128 * (mf + 1)],
                        rhs=out1T_sb[:, k, 512 * c : 512 * (c + 1)],
                        start=(k == 0),
                        stop=(k == KT - 1),
                    )
                nc.scalar.activation(
                    out=ht[:, 512 * c : 512 * (c + 1)],
                    in_=ps[:],
                    func=AF.Relu,
                    bias=b1_sb[:, mf : mf + 1],
                    scale=1.0,
                )

        # ---- phase 5: FFN second layer + residual + LN2 + store ----
        for m in range(ST):
            res = strp.tile([128, D], f32, name="res2", tag="res")
            for n0 in range(2):
                ps = psA.tile([128, 384], f32, name="ps_mm", tag="mm")
                for k in range(FT):
                    nc.tensor.matmul(
                        ps[:],
                        lhsT=hts[k][:, 128 * m : 128 * (m + 1)],
                        rhs=w2_sbs[k // 8][:, k % 8, 384 * n0 : 384 * (n0 + 1)],
                        start=(k == 0),
                        stop=(k == FT - 1),
                    )
                nc.vector.tensor_add(
                    out=res[:, 384 * n0 : 384 * (n0 + 1)],
                    in0=ps[:],
                    in1=out1_sb[:, m, 384 * n0 : 384 * (n0 + 1)],
                )
            nc.gpsimd.tensor_add(out=res[:], in0=res[:], in1=b2_bc[:])
            out2 = strp.tile([128, D], f32, name="out2", tag="res")
            emit_ln(res, res, out2[:], g2_bc, be2_bc)
            nc.sync.dma_start(out=out_d[128 * m : 128 * (m + 1), :], in_=out2[:])

    nc.compile()
    return nc


def _get_nc():
    if "nc" not in _BUILD_CACHE:
        _BUILD_CACHE["nc"] = _build_nc()
    return _BUILD_CACHE["nc"]


def _make_in_maps(inputs):
    x = np.asarray(inputs["x"], dtype=np.float32)
    tri = np.triu(np.ones((128, 128), dtype=np.float32)).astype(BF16)
    ident = np.eye(128, dtype=np.float32).astype(BF16)

    shared = {
        "wq": np.asarray(inputs["wq"], np.float32).astype(BF16),
        "wk": np.asarray(inputs["wk"], np.float32).astype(BF16),
        "wv": np.asarray(inputs["wv"], np.float32).astype(BF16),
        "wo": np.asarray(inputs["wo"], np.float32).astype(BF16),
        "w1": np.asarray(inputs["w1"], np.float32).astype(BF16),
        "w2": np.asarray(inputs["w2"], np.float32).astype(BF16),
        "bq": np.asarray(inputs["bq"], np.float32),
        "bk": np.asarray(inputs["bk"], np.float32),
        "b1": np.asarray(inputs["b1"], np.float32),
        "bv": np.asarray(inputs["bv"], np.float32).astype(BF16),
        "bo": np.asarray(inputs["bo"], np.float32).astype(BF16),
        "b2": np.asarray(inputs["b2"], np.float32).astype(BF16),
        "g1": np.asarray(inputs["g1"], np.float32).astype(BF16),
        "be1": np.asarray(inputs["be1"], np.float32).astype(BF16),
        "g2": np.asarray(inputs["g2"], np.float32).astype(BF16),
        "be2": np.asarray(inputs["be2"], np.float32).astype(BF16),
        "trimask": tri,
        "identb": ident,
    }
    in_maps = []
    for b in range(N_CORES):
        m = dict(shared)
        m["x"] = np.ascontiguousarray(x[b])
        m["xT"] = np.ascontiguousarray(x[b].T.astype(BF16))
        in_maps.append(m)
    return in_maps


def _run(inputs, trace=False):
    from concourse.bass_utils import run_bass_kernel_spmd

    nc = _get_nc()
    in_maps = _make_in_maps(inputs)
    res = run_bass_kernel_spmd(
        nc, in_maps, core_ids=list(range(N_CORES)), trace=trace
    )
    out = np.stack([res.results[b]["out"] for b in range(N_CORES)], axis=0)
    return out.astype(np.float32), res


def kernel(**inputs):
    out, _ = _run(inputs, trace=False)
    return out


# ---------------------------------------------------------------------------
# Timing support (used by test.py; measures device execution time by running
# the compiled NEFF repeatedly with device-resident inputs and subtracting
# the dispatch overhead of an empty kernel measured the same way).
# ---------------------------------------------------------------------------


def _make_sharded_callable(nc, n_cores=N_CORES):
    import jax
    from jax.sharding import Mesh, NamedSharding, PartitionSpec
    from jax.experimental.shard_map import shard_map
    from concourse import bass2jax, mybir

    bass2jax.install_neuronx_cc_hook()

    partition_name = (
        nc.partition_id_tensor.name if nc.partition_id_tensor else None
    )
    in_names, out_names, out_avals, zero_outs = [], [], [], []
    for alloc in nc.m.functions[0].allocations:
        if not isinstance(alloc, mybir.MemoryLocationSet):
            continue
        name = alloc.memorylocations[0].name
        if alloc.kind == "ExternalInput":
            if name != partition_name:
                in_names.append(name)
        elif alloc.kind == "ExternalOutput":
            shape = tuple(alloc.tensor_shape)
            dtype = mybir.dt.np(alloc.dtype)
            out_names.append(name)
            out_avals.append(jax.core.ShapedArray(shape, dtype))
            zero_outs.append(np.zeros(shape, dtype))
    n_params = len(in_names)
    all_in_names = list(in_names) + list(out_names)
    if partition_name is not None:
        all_in_names.append(partition_name)

    def _body(*args):
        operands = list(args)
        if partition_name is not None:
            operands.append(bass2jax.partition_id_tensor())
        outs = bass2jax._bass_exec_p.bind(
            *operands,
            out_avals=tuple(out_avals),
            in_names=tuple(all_in_names),
            out_names=tuple(out_names),
            lowering_input_output_aliases=(),
            sim_require_finite=True,
            sim_require_nnan=True,
            nc=nc,
        )
        return tuple(outs)

    devices = jax.devices()[:n_cores]
    mesh = Mesh(np.asarray(devices), ("core",))
    n_outs = len(out_avals)
    in_specs = (PartitionSpec("core"),) * (n_params + n_outs)
    out_specs = (PartitionSpec("core"),) * n_outs
    fn = jax.jit(
        shard_map(
            _body, mesh=mesh, in_specs=in_specs, out_specs=out_specs,
            check_rep=False,
        ),
        keep_unused=True,
    )
    sharding = NamedSharding(mesh, PartitionSpec("core"))
    return fn, in_names, zero_outs, sharding


def _hw_time_ns(inputs, iters=20):
    """Median wall time per sharded NEFF execution, with device-resident
    inputs, minus the same measurement for a trivial kernel (dispatch
    overhead)."""
    import time as _time

    import jax

    nc = _get_nc()
    fn, in_names, zero_outs, sharding = _make_sharded_callable(nc)
    in_maps = _make_in_maps(inputs)
    n_cores = len(in_maps)
    dev_args = []
    for name in in_names:
        cat = np.concatenate([np.asarray(m[name]) for m in in_maps], axis=0)
        dev_args.append(jax.device_put(cat, sharding))
    for z in zero_outs:
        cat = np.zeros((n_cores * z.shape[0], *z.shape[1:]), z.dtype)
        dev_args.append(jax.device_put(cat, sharding))

    def timed(f, args, n):
        jax.block_until_ready(f(*args))  # warmup / compile
        ts = []
        for _ in range(n):
            t0 = _time.perf_counter()
            jax.block_until_ready(f(*args))
            ts.append(_time.perf_counter() - t0)
        ts.sort()
        return ts[len(ts) // 2], ts

    t_main, ts_main = timed(fn, dev_args, iters)

    # trivial baseline kernel: single small DMA per core
    if "baseline" not in _BUILD_CACHE:
        from contextlib import ExitStack

        import concourse.tile as tile
        from concourse import bacc, mybir

        f32 = mybir.dt.float32
        bnc = bacc.Bacc(
            "TRN2", target_bir_lowering=False, debug=False, num_devices=N_CORES
        )
        a_d = bnc.declare_dram_parameter("a", [128, 128], f32, isOutput=False)
        o_d = bnc.declare_dram_parameter("o", [128, 128], f32, isOutput=True)
        with ExitStack() as ctx:
            tc = ctx.enter_context(tile.TileContext(bnc))
            pool = ctx.enter_context(tc.tile_pool(name="p", bufs=1))
            t = pool.tile([128, 128], f32, name="t")
            bnc.sync.dma_start(out=t[:], in_=a_d[:])
            bnc.sync.dma_start(out=o_d[:], in_=t[:])
        bnc.compile()
        _BUILD_CACHE["baseline"] = bnc
    bnc = _BUILD_CACHE["baseline"]
    bfn, bin_names, bzeros, bshard = _make_sharded_callable(bnc)
    a = np.zeros((N_CORES * 128, 128), np.float32)
    bargs = [jax.device_put(a, bshard)]
    for z in bzeros:
        bargs.append(
            jax.device_put(np.zeros((N_CORES * z.shape[0], *z.shape[1:]), z.dtype), bshard)
        )
    t_base, ts_base = timed(bfn, bargs, iters)

    return {
        "exec_ns": (t_main - t_base) * 1e9,
        "raw_ns": t_main * 1e9,
        "base_ns": t_base * 1e9,
        "all_main": ts_main,
        "all_base": ts_base,
    }
